# revision 1
# baseline (speedup 1.0000x reference)
"""Trainium2 Bass kernel for nn_EuclideanToLorentzConv (8-core data-parallel).

Pipeline (per core, batch shard of 2 images):
  1. Lorentz conv as 9 window-matmuls, K=126 packing [s | s^2]; the s^2 half
     carries ones-weights into output channel 127, accumulating T^2-1.
  2. Channel-major y' lives in SBUF; per-pixel scalar fields (T, sum y^2, dot
     products) are extracted via small PE reductions + reshape DMAs into a
     [56, 448] pixel-major layout where the transcendental chains run at full
     lane utilization.
  3. Lorentz batchnorm statistics via two tiny AllReduces (mu vector, var).
  4. tmp = y' + W0*T - mu_s*H is formed with one K=2 rank-1 matmul + one DVE
     add; the output pass fuses relu+scale in one scalar_tensor_tensor.
All heavy per-element math is fp32.
"""

import sys
import numpy as np
from contextlib import ExitStack

sys.path.insert(0, "/opt/trn_rl_repo")

import concourse.bass as bass  # noqa: E402
import concourse.tile as tile  # noqa: E402
from concourse import mybir, bacc  # noqa: E402
from concourse.bass_utils import run_bass_kernel_spmd  # noqa: E402

F32 = mybir.dt.float32
AX = mybir.AxisListType
OP = mybir.AluOpType
AF = mybir.ActivationFunctionType

# ---- problem constants (hardcoded; kernel.py must be self-contained) ----
NCORES = 8
B_GLOB, CIN, H, W = 16, 64, 112, 112
B_LOC = B_GLOB // NCORES            # 2 images per core
S = CIN - 1                         # 63 space channels in
M = 127                             # space channels out
COUT = M + 1
D = 9 * S + 1                       # 568
EPS = 1e-6

HP, WP = H + 2, W + 2               # padded
ROWS_PER_GROUP = 4
GROUP_PX = ROWS_PER_GROUP * W       # 448
BAND_ROWS = 16                      # output rows per band
GROUPS_PER_BAND = BAND_ROWS // ROWS_PER_GROUP   # 4
BANDS_PER_IMG = H // BAND_ROWS      # 7
NBANDS = B_LOC * BANDS_PER_IMG      # 14
NGROUPS = NBANDS * GROUPS_PER_BAND  # 56
NPX = NGROUPS * GROUP_PX            # 25088 pixels per core
NPX_GLOB = B_GLOB * H * W           # 200704
SPAD_ROWS = BAND_ROWS + 2           # 18 padded rows per band buffer

_CACHE = {}


def _build_nc():
    nc = bacc.Bacc("TRN2", target_bir_lowering=False, debug=False,
                   num_devices=NCORES)

    x_in = nc.dram_tensor("x", [B_LOC, CIN, H, W], F32, kind="ExternalInput")
    w9_in = nc.dram_tensor("w9", [128, 9 * 128], F32, kind="ExternalInput")
    redw_in = nc.dram_tensor("redw", [M, 3], F32, kind="ExternalInput")
    lr1_in = nc.dram_tensor("lr1i", [2, 128], F32, kind="ExternalInput")
    gamma_in = nc.dram_tensor("gamma", [1], F32, kind="ExternalInput")
    out_d = nc.dram_tensor("out", [B_LOC, COUT, H, W], F32,
                           kind="ExternalOutput")

    if _CACHE.get("debug"):
        dbg_ycm = nc.dram_tensor("dbg_ycm", [128, NPX], F32, kind="ExternalOutput")
        dbg_ps1 = nc.dram_tensor("dbg_ps1", [5, NGROUPS, GROUP_PX], F32, kind="ExternalOutput")
        dbg_ps2 = nc.dram_tensor("dbg_ps2", [6, NGROUPS, GROUP_PX], F32, kind="ExternalOutput")
        dbg_mu = nc.dram_tensor("dbg_mu", [130], F32, kind="ExternalOutput")
        dbg_tmp = nc.dram_tensor("dbg_tmp", [128, NPX], F32, kind="ExternalOutput")
    cc1_in = nc.dram_tensor("cc1_in", [130], F32)
    cc1_out = nc.dram_tensor("cc1_out", [130], F32, addr_space="Shared")
    cc2_in = nc.dram_tensor("cc2_in", [2], F32)
    cc2_out = nc.dram_tensor("cc2_out", [2], F32, addr_space="Shared")
    groups_all = [list(range(NCORES))]

    with tile.TileContext(nc) as tc, ExitStack() as ctx:
        sing = ctx.enter_context(tc.tile_pool(name="sing", bufs=1))
        spadp = ctx.enter_context(tc.tile_pool(name="spad", bufs=2))
        scrp = ctx.enter_context(tc.tile_pool(name="scr", bufs=2))
        outp = ctx.enter_context(tc.tile_pool(name="outp", bufs=3))
        stgp = ctx.enter_context(tc.tile_pool(name="stg", bufs=2))
        psy = ctx.enter_context(tc.tile_pool(name="psy", bufs=3, space="PSUM"))
        pss = ctx.enter_context(tc.tile_pool(name="pss", bufs=3, space="PSUM"))
        pst = ctx.enter_context(tc.tile_pool(name="pst", bufs=1, space="PSUM"))

        # ---- static SBUF ----
        W9 = sing.tile([128, 9, 128], F32)
        nc.sync.dma_start(out=W9, in_=w9_in[:].rearrange("p (w m) -> p w m", w=9))
        REDW = sing.tile([M, 3], F32)
        nc.sync.dma_start(out=REDW, in_=redw_in[:])
        LR1 = sing.tile([2, 128], F32)
        nc.sync.dma_start(out=LR1, in_=lr1_in[:])
        GAM = sing.tile([1, 1], F32)
        nc.sync.dma_start(out=GAM, in_=gamma_in[:].rearrange("(o c) -> o c", o=1))
        ONES56 = sing.tile([56, 1], F32)
        nc.vector.memset(ONES56, 1.0)
        BYT = sing.tile([56, 1], F32)
        nc.vector.memset(BYT, float(1.0 + _CACHE["c_w0sq"]))
        BM1 = sing.tile([56, 1], F32)
        nc.vector.memset(BM1, -1.0)
        BEPSV = sing.tile([1, 1], F32)
        nc.vector.memset(BEPSV, 1e-5)

        YCM = sing.tile([128, NPX], F32)          # rows 0..126 y', row 127 T^2-1
        MUP = sing.tile([128, NGROUPS], F32)      # per-group per-channel sums

        # pixel-scalar fields, [56, 448] (partition = group)
        def ps(name):
            t = sing.tile([NGROUPS, GROUP_PX], F32, tag=name)
            return t
        T2M1, TPS, W0DOT, YSQ1, YT = ps("t2m1"), ps("tps"), ps("w0dot"), ps("ysq1"), ps("yt")
        MUDOT, ALPHA, FPS, HPS = ps("mudot"), ps("alpha"), ps("fps"), ps("hps")
        STSQ, RSQ2, PSA, PSB, PSC = ps("stsq"), ps("rsq2"), ps("psa"), ps("psb"), ps("psc")

        # ================= PHASE 1: conv =================
        for band in range(NBANDS):
            b, rb = divmod(band, BANDS_PER_IMG)
            r_lo = rb * BAND_ROWS - 1                    # first padded input row
            SPAD = spadp.tile([128, SPAD_ROWS, WP], F32, tag="spad")
            nc.gpsimd.memset(SPAD[:], 0.0)
            # input rows r_lo .. r_lo+17 clipped to [0, 112)
            src_lo = max(r_lo, 0)
            src_hi = min(r_lo + SPAD_ROWS, H)
            d_lo = src_lo - r_lo
            d_hi = d_lo + (src_hi - src_lo)
            for base in (0, 64):
                nc.sync.dma_start(
                    out=SPAD[base:base + 63, d_lo:d_hi, 1:WP - 1],
                    in_=x_in[b, 1:CIN, src_lo:src_hi, :])
            # square the second copy in place
            nc.scalar.activation(out=SPAD[64:127, :, :], in_=SPAD[64:127, :, :],
                                 func=AF.Square)

            STG_AB = stgp.tile([2, GROUPS_PER_BAND, GROUP_PX], F32, tag="stgx")
            for k in range(GROUPS_PER_BAND):
                g = band * GROUPS_PER_BAND + k
                cols = bass.ts(g, GROUP_PX)
                R = k * ROWS_PER_GROUP                  # band-local out row
                psum = psy.tile([128, GROUP_PX], F32, tag="psy")
                for wi in range(9):
                    i, j = divmod(wi, 3)
                    rhs = SPAD[:, R + i:R + i + ROWS_PER_GROUP, j:j + W]
                    nc.tensor.matmul(psum[:], lhsT=W9[:, wi, :], rhs=rhs,
                                     start=(wi == 0), stop=(wi == 8))
                # evacuate + per-channel partial sums (for mu)
                nc.vector.tensor_scalar(out=YCM[:, cols], in0=psum[:],
                                        scalar1=0.0, scalar2=None, op0=OP.add,
                                        op1=OP.add, accum_out=MUP[:, g:g + 1])
                # squared copy for sum_c y'^2
                ysq_t = scrp.tile([M, GROUP_PX], F32, tag="ysqscr")
                nc.scalar.activation(out=ysq_t, in_=psum[0:M, :], func=AF.Square)
                ps2 = pss.tile([2, GROUP_PX], F32, tag="pss")
                nc.tensor.matmul(ps2[0:2, :], lhsT=REDW[:, 0:2],
                                 rhs=YCM[0:M, cols], start=True, stop=False)
                nc.tensor.matmul(ps2[0:2, :], lhsT=REDW[:, 1:3],
                                 rhs=ysq_t[:], start=False, stop=True)
                nc.vector.tensor_copy(out=STG_AB[:, k, :], in_=ps2[:])
            gsl = bass.ts(band, GROUPS_PER_BAND)
            csl = bass.ts(band, GROUPS_PER_BAND * GROUP_PX)
            nc.sync.dma_start(out=W0DOT[gsl, :], in_=STG_AB[0:1, :, :])
            nc.sync.dma_start(out=YSQ1[gsl, :], in_=STG_AB[1:2, :, :])
            nc.sync.dma_start(out=T2M1[gsl, :], in_=YCM[127:128, csl])

        # ---- pixel-scalar chain, phase 1 ----
        C_W0SQ = None  # placed via host in redw? no: computed host-side as imm
        # T = sqrt(1 + T2m1)
        nc.scalar.activation(out=TPS, in_=T2M1, func=AF.Sqrt, bias=1.0)
        # ysqf = ysq1 + 2*T*w0dot + T2m1*c_w0sq ; y_t = sqrt(1 + c_w0sq + ysqf')
        nc.vector.tensor_mul(PSA, TPS, W0DOT)
        nc.vector.scalar_tensor_tensor(out=PSB, in0=PSA, scalar=2.0, in1=YSQ1,
                                       op0=OP.mult, op1=OP.add)
        # PSC = T2m1*c_w0sq + PSB   (c_w0sq patched below via immediate)
        nc.vector.scalar_tensor_tensor(out=PSC, in0=T2M1, scalar=_CACHE["c_w0sq"],
                                       in1=PSB, op0=OP.mult, op1=OP.add)
        nc.scalar.activation(out=YT, in_=PSC, func=AF.Sqrt, bias=BYT[:])
        # reduced sums for the collective
        MUS = sing.tile([128, 1], F32)
        nc.vector.tensor_reduce(MUS, MUP, axis=AX.X, op=OP.add)
        SR = sing.tile([56, 2], F32)
        nc.vector.tensor_reduce(SR[:, 0:1], TPS, axis=AX.X, op=OP.add)
        nc.vector.tensor_reduce(SR[:, 1:2], YT, axis=AX.X, op=OP.add)
        pt = pst.tile([1, 8], F32, tag="pst")
        nc.tensor.matmul(pt[0:1, 0:2], lhsT=ONES56, rhs=SR[:], start=True, stop=True)
        SC0 = sing.tile([1, 2], F32)
        nc.vector.tensor_copy(out=SC0, in_=pt[0:1, 0:2])
        nc.sync.dma_start(out=cc1_in[0:128], in_=MUS)
        nc.sync.dma_start(out=cc1_in[128:130], in_=SC0)
        nc.gpsimd.collective_compute("AllReduce", OP.add, replica_groups=groups_all,
                                     ins=[cc1_in[:]], outs=[cc1_out[:]])
        MUSG = sing.tile([128, 1], F32)
        nc.sync.dma_start(out=MUSG, in_=cc1_out[0:128].rearrange("(p o) -> p o", o=1))
        SC0G = sing.tile([1, 2], F32)
        nc.sync.dma_start(out=SC0G, in_=cc1_out[128:130].rearrange("(o c) -> o c", o=1))

        # ---- mu normalization (tiny ops) ----
        invN = 1.0 / float(NPX_GLOB)
        SC127 = sing.tile([M, 2], F32)
        nc.gpsimd.partition_broadcast(SC127, SC0G)
        MUUS = sing.tile([M, 1], F32)      # unnormalized mean of y_s
        nc.vector.scalar_tensor_tensor(out=MUUS, in0=REDW[:, 0:1],
                                       scalar=SC127[:, 0:1], in1=MUSG[0:M, :],
                                       op0=OP.mult, op1=OP.add)
        nc.vector.tensor_scalar_mul(MUUS, MUUS, invN)
        MU0U = sing.tile([1, 1], F32)
        nc.vector.tensor_scalar_mul(MU0U, SC0G[0:1, 1:2], invN)
        MSQ = sing.tile([M, 1], F32)
        nc.vector.tensor_mul(MSQ, MUUS, MUUS)
        pt2 = pst.tile([1, 8], F32, tag="pst")
        nc.tensor.matmul(pt2[0:1, 0:1], lhsT=REDW[:, 2:3], rhs=MSQ[:],
                         start=True, stop=True)
        SMSQ = sing.tile([1, 1], F32)
        nc.vector.tensor_copy(out=SMSQ, in_=pt2[0:1, 0:1])
        T1 = sing.tile([1, 1], F32)
        nc.vector.tensor_mul(T1, MU0U, MU0U)
        nc.vector.tensor_sub(T1, T1, SMSQ)
        nc.scalar.activation(out=T1, in_=T1, func=AF.Sqrt)     # nrm
        RNRM = sing.tile([1, 1], F32)
        nc.vector.reciprocal(RNRM, T1)
        RN127 = sing.tile([M, 1], F32)
        nc.gpsimd.partition_broadcast(RN127, RNRM)
        MUHS = sing.tile([M, 1], F32)
        nc.vector.tensor_scalar_mul(MUHS, MUUS, RN127[:, 0:1])
        MU0H = sing.tile([1, 1], F32)
        nc.vector.tensor_mul(MU0H, MU0U, RNRM)
        # c_muW0 = sum(mu_s * W0)
        PRD = sing.tile([M, 1], F32)
        nc.vector.tensor_mul(PRD, MUHS, REDW[:, 0:1])
        pt3 = pst.tile([1, 8], F32, tag="pst")
        nc.tensor.matmul(pt3[0:1, 0:1], lhsT=REDW[:, 2:3], rhs=PRD[:],
                         start=True, stop=True)
        # inv1p = 1/(1+mu0)
        INV1P = sing.tile([1, 1], F32)
        nc.vector.tensor_scalar_add(INV1P, MU0H, 1.0)
        nc.vector.reciprocal(INV1P, INV1P)
        # scalar bundle -> 56 partitions: {mu0, inv1p, c_muW0}
        SCROW = sing.tile([1, 4], F32)
        nc.vector.tensor_copy(out=SCROW[:, 0:1], in_=MU0H)
        nc.vector.tensor_copy(out=SCROW[:, 1:2], in_=INV1P)
        nc.vector.tensor_copy(out=SCROW[:, 2:3], in_=pt3[0:1, 0:1])
        SC56 = sing.tile([56, 4], F32)
        nc.gpsimd.partition_broadcast(SC56, SCROW)
        # LR1 row0 = -mu_s  (tiny transposing DMA [127,1] -> [1,127])
        NMU = sing.tile([M, 1], F32)
        nc.vector.tensor_scalar_mul(NMU, MUHS, -1.0)
        nc.sync.dma_start(out=LR1[0:1, 0:M], in_=NMU[:])

        if _CACHE.get("debug"):
            nc.sync.dma_start(out=dbg_ycm[:], in_=YCM[:])
            for i_, t_ in enumerate([T2M1, TPS, W0DOT, YSQ1, YT]):
                nc.sync.dma_start(out=dbg_ps1[i_], in_=t_[:])
            nc.sync.dma_start(out=dbg_mu[0:128], in_=MUSG[:])
            nc.sync.dma_start(out=dbg_mu[128:130], in_=SC0G[:])

        # ================= PHASE 2 =================
        for band in range(NBANDS):
            STG_C = stgp.tile([1, GROUPS_PER_BAND, GROUP_PX], F32, tag="stgc")
            for k in range(GROUPS_PER_BAND):
                g = band * GROUPS_PER_BAND + k
                cols = bass.ts(g, GROUP_PX)
                ps2 = pss.tile([2, GROUP_PX], F32, tag="pss")
                nc.tensor.matmul(ps2[0:1, :], lhsT=MUHS, rhs=YCM[0:M, cols],
                                 start=True, stop=True)
                nc.vector.tensor_copy(out=STG_C[:, k, :], in_=ps2[0:1, :])
            nc.sync.dma_start(out=MUDOT[bass.ts(band, GROUPS_PER_BAND), :], in_=STG_C[0:1, :, :])

        # alpha = clip(mu0*yt - (mudot + T*c_muW0), 1+eps)
        nc.vector.scalar_tensor_tensor(out=PSA, in0=TPS, scalar=SC56[:, 2:3],
                                       in1=MUDOT, op0=OP.mult, op1=OP.add)
        nc.vector.tensor_scalar(out=PSB, in0=YT, scalar1=SC56[:, 0:1],
                                scalar2=None, op0=OP.mult)
        nc.vector.tensor_sub(ALPHA, PSB, PSA)
        nc.vector.tensor_scalar_max(ALPHA, ALPHA, 1.0 + EPS)
        # f = ln(alpha + sqrt(alpha^2-1)) / sqrt(alpha^2-1)
        nc.vector.tensor_mul(PSA, ALPHA, ALPHA)
        nc.scalar.activation(out=PSB, in_=PSA, func=AF.Sqrt, bias=BM1[:])
        nc.vector.tensor_add(PSC, ALPHA, PSB)
        nc.scalar.activation(out=PSC, in_=PSC, func=AF.Ln)
        nc.vector.reciprocal(PSB, PSB)
        nc.vector.tensor_mul(FPS, PSC, PSB)
        # H = alpha + (yt - alpha*mu0) * inv1p
        nc.vector.tensor_scalar(out=PSA, in0=ALPHA, scalar1=SC56[:, 0:1],
                                scalar2=None, op0=OP.mult)
        nc.vector.tensor_sub(PSA, YT, PSA)
        nc.vector.scalar_tensor_tensor(out=HPS, in0=PSA, scalar=SC56[:, 1:2],
                                       in1=ALPHA, op0=OP.mult, op1=OP.add)

        # tmp = y' + W0*T - mu_s*H ; stsq = sum_c tmp^2
        for band in range(NBANDS):
            gsl = bass.ts(band, GROUPS_PER_BAND)
            HT = stgp.tile([2, GROUPS_PER_BAND, GROUP_PX], F32, tag="stgx")
            nc.sync.dma_start(out=HT[0:1, :, :], in_=HPS[gsl, :])
            nc.sync.dma_start(out=HT[1:2, :, :], in_=TPS[gsl, :])
            STG_C = stgp.tile([1, GROUPS_PER_BAND, GROUP_PX], F32, tag="stgc")
            for k in range(GROUPS_PER_BAND):
                g = band * GROUPS_PER_BAND + k
                cols = bass.ts(g, GROUP_PX)
                pr1 = psy.tile([128, GROUP_PX], F32, tag="psy")
                nc.tensor.matmul(pr1[:], lhsT=LR1, rhs=HT[:, k, :],
                                 start=True, stop=True)
                nc.vector.tensor_add(YCM[:, cols], YCM[:, cols], pr1[:])
                sq_t = scrp.tile([M, GROUP_PX], F32, tag="ysqscr")
                nc.scalar.activation(out=sq_t, in_=YCM[0:M, cols], func=AF.Square)
                ps2 = pss.tile([2, GROUP_PX], F32, tag="pss")
                nc.tensor.matmul(ps2[0:1, :], lhsT=REDW[:, 2:3], rhs=sq_t[:],
                                 start=True, stop=True)
                nc.vector.tensor_copy(out=STG_C[:, k, :], in_=ps2[0:1, :])
            nc.sync.dma_start(out=STSQ[bass.ts(band, GROUPS_PER_BAND), :], in_=STG_C[0:1, :, :])

        if _CACHE.get("debug"):
            nc.sync.dma_start(out=dbg_tmp[:], in_=YCM[:])
            for i_, t_ in enumerate([MUDOT, ALPHA, FPS, HPS, STSQ]):
                nc.sync.dma_start(out=dbg_ps2[i_], in_=t_[:])

        # var = mean(f^2 * stsq)  -> allreduce
        nc.vector.tensor_mul(PSA, FPS, FPS)
        nc.vector.tensor_mul(PSB, PSA, STSQ)
        VR = sing.tile([56, 1], F32)
        nc.vector.tensor_reduce(VR, PSB, axis=AX.X, op=OP.add)
        pt4 = pst.tile([1, 8], F32, tag="pst")
        nc.tensor.matmul(pt4[0:1, 0:1], lhsT=ONES56, rhs=VR[:], start=True, stop=True)
        VSC = sing.tile([1, 2], F32)
        nc.vector.tensor_copy(out=VSC[:, 0:1], in_=pt4[0:1, 0:1])
        nc.vector.tensor_copy(out=VSC[:, 1:2], in_=pt4[0:1, 0:1])
        nc.sync.dma_start(out=cc2_in[:], in_=VSC)
        nc.gpsimd.collective_compute("AllReduce", OP.add, replica_groups=groups_all,
                                     ins=[cc2_in[:]], outs=[cc2_out[:]])
        VG = sing.tile([1, 2], F32)
        nc.sync.dma_start(out=VG, in_=cc2_out[:].rearrange("(o c) -> o c", o=1))
        GSC = sing.tile([1, 1], F32)
        nc.vector.tensor_scalar_mul(GSC, VG[0:1, 0:1], invN)
        nc.scalar.activation(out=GSC, in_=GSC, func=AF.Sqrt, bias=BEPSV[:])
        nc.vector.reciprocal(GSC, GSC)
        nc.vector.tensor_mul(GSC, GSC, GAM)
        G56 = sing.tile([56, 1], F32)
        nc.gpsimd.partition_broadcast(G56, GSC)

        # ================= PHASE 3 =================
        # gf = g*f ; vn = sqrt(max(gf^2*stsq, eps)); w2 = gf*sinh(vn)/vn
        nc.vector.tensor_scalar(out=PSA, in0=FPS, scalar1=G56[:, 0:1],
                                scalar2=None, op0=OP.mult)          # gf
        nc.vector.tensor_mul(PSB, PSA, PSA)
        nc.vector.tensor_mul(PSB, PSB, STSQ)
        nc.vector.tensor_scalar_max(PSB, PSB, EPS)
        VN = TPS
        nc.scalar.activation(out=VN, in_=PSB, func=AF.Sqrt)
        EX = W0DOT
        nc.scalar.activation(out=EX, in_=VN, func=AF.Exp)
        EIX = YSQ1
        nc.vector.reciprocal(EIX, EX)
        nc.vector.tensor_sub(PSB, EX, EIX)                          # 2*sinh
        nc.vector.reciprocal(PSC, VN)
        nc.vector.tensor_mul(PSB, PSB, PSC)
        nc.vector.tensor_scalar_mul(PSB, PSB, 0.5)                  # sinh/vn
        W2 = MUDOT
        nc.vector.tensor_mul(W2, PSA, PSB)
        if _CACHE.get("debug"):
            nc.sync.dma_start(out=dbg_ps2[5], in_=W2[:])

        for band in range(NBANDS):
            b, rb = divmod(band, BANDS_PER_IMG)
            gsl = bass.ts(band, GROUPS_PER_BAND)
            W2S = stgp.tile([1, GROUPS_PER_BAND, GROUP_PX], F32, tag="stgx")
            nc.sync.dma_start(out=W2S[0:1, :, :], in_=W2[gsl, :])
            STG_C = stgp.tile([1, GROUPS_PER_BAND, GROUP_PX], F32, tag="stgc")
            for k in range(GROUPS_PER_BAND):
                g = band * GROUPS_PER_BAND + k
                cols = bass.ts(g, GROUP_PX)
                w2r = outp.tile([M, GROUP_PX], F32, tag="w2r")
                nc.gpsimd.partition_broadcast(w2r, W2S[0:1, k, :])
                outs = outp.tile([M, GROUP_PX], F32, tag="outs")
                nc.vector.scalar_tensor_tensor(out=outs, in0=YCM[0:M, cols],
                                               scalar=0.0, in1=w2r,
                                               op0=OP.max, op1=OP.mult)
                row0 = rb * BAND_ROWS + k * ROWS_PER_GROUP
                nc.sync.dma_start(
                    out=out_d[b, 1:COUT, row0:row0 + ROWS_PER_GROUP, :],
                    in_=outs[:].rearrange("p (r c) -> p r c", r=ROWS_PER_GROUP))
                sq_o = scrp.tile([M, GROUP_PX], F32, tag="ysqscr")
                nc.scalar.activation(out=sq_o, in_=outs, func=AF.Square)
                ps2 = pss.tile([2, GROUP_PX], F32, tag="pss")
                nc.tensor.matmul(ps2[0:1, :], lhsT=REDW[:, 2:3], rhs=sq_o[:],
                                 start=True, stop=True)
                nc.vector.tensor_copy(out=STG_C[:, k, :], in_=ps2[0:1, :])
            nc.sync.dma_start(out=RSQ2[gsl, :], in_=STG_C[0:1, :, :])

        # rt = sqrt(1 + sum rs^2) -> channel 0 plane
        RT = T2M1
        nc.scalar.activation(out=RT, in_=RSQ2, func=AF.Sqrt, bias=1.0)
        nc.sync.dma_start(out=out_d[:, 0, :, :], in_=RT)

    nc.compile()
    return nc


def _prep_consts(W):
    W = np.asarray(W, np.float32)
    w9 = np.zeros((128, 9, 128), np.float32)
    for wi in range(9):
        w9[0:S, wi, 0:M] = W[:, 1 + wi * S:1 + (wi + 1) * S].T
        w9[64:64 + S, wi, 127] = 1.0
    redw = np.zeros((M, 3), np.float32)
    redw[:, 0] = W[:, 0]
    redw[:, 2] = 1.0
    lr1 = np.zeros((2, 128), np.float32)
    lr1[1, 0:M] = W[:, 0]
    c_w0sq = float(np.float32((W[:, 0].astype(np.float64) ** 2).sum()))
    return w9.reshape(128, 9 * 128), redw, lr1, c_w0sq


def kernel(x, W, gamma, beta):
    x = np.ascontiguousarray(np.asarray(x, np.float32))
    W = np.asarray(W, np.float32)
    gamma = np.asarray(gamma, np.float32)
    beta = np.asarray(beta, np.float32)
    assert abs(float(beta[0]) - 1.0) < 1e-6 and np.all(np.abs(beta[1:]) < 1e-6), \
        "kernel specialized for beta == Lorentz origin"
    assert float(gamma[0]) > 0.0

    w9, redw, lr1, c_w0sq = _prep_consts(W)
    if "nc" not in _CACHE:
        _CACHE["c_w0sq"] = c_w0sq
        _CACHE["nc"] = _build_nc()
    nc = _CACHE["nc"]

    in_maps = []
    for c in range(NCORES):
        in_maps.append({
            "x": x[c * B_LOC:(c + 1) * B_LOC],
            "w9": w9, "redw": redw, "lr1i": lr1,
            "gamma": gamma,
        })
    res = run_bass_kernel_spmd(nc, in_maps, list(range(NCORES)))
    out = np.concatenate([res.results[c]["out"] for c in range(NCORES)], axis=0)
    return out


def run_traced(inputs, tmpdir=None):
    """Run with NTFF tracing; returns (exec_time_ns, BassKernelResults)."""
    x = np.ascontiguousarray(np.asarray(inputs["x"], np.float32))
    w9, redw, lr1, c_w0sq = _prep_consts(inputs["W"])
    if "nc" not in _CACHE:
        _CACHE["c_w0sq"] = c_w0sq
        _CACHE["nc"] = _build_nc()
    nc = _CACHE["nc"]
    in_maps = []
    for c in range(NCORES):
        in_maps.append({
            "x": x[c * B_LOC:(c + 1) * B_LOC],
            "w9": w9, "redw": redw, "lr1i": lr1,
            "gamma": np.asarray(inputs["gamma"], np.float32),
        })
    res = run_bass_kernel_spmd(nc, in_maps, list(range(NCORES)),
                               trace=True, tmpdir=tmpdir)
    return res.exec_time_ns, res


def simulate(inputs, debug=True):
    """Run the kernel through MultiCoreSim; returns list of per-core output dicts."""
    from concourse.bass_interp import MultiCoreSim
    _CACHE.clear()
    _CACHE["debug"] = debug
    x = np.asarray(inputs["x"], np.float32)
    w9, redw, lr1, c_w0sq = _prep_consts(inputs["W"])
    _CACHE["c_w0sq"] = c_w0sq
    nc = _build_nc()
    sim = MultiCoreSim(nc, num_cores=NCORES)
    for c in range(NCORES):
        cs = sim.cores[c]
        cs.tensor("x")[:] = x[c * B_LOC:(c + 1) * B_LOC]
        cs.tensor("w9")[:] = w9
        cs.tensor("redw")[:] = redw
        cs.tensor("lr1i")[:] = lr1
        cs.tensor("gamma")[:] = np.asarray(inputs["gamma"], np.float32)
    sim.simulate(check_with_hw=False)
    names = ["out"]
    if debug:
        names += ["dbg_ycm", "dbg_ps1", "dbg_ps2", "dbg_mu", "dbg_tmp"]
    return [{n: np.array(sim.cores[c].tensor(n)) for n in names}
            for c in range(NCORES)]


if __name__ == "__main__":
    rng = np.random.default_rng(0)
    x = rng.standard_normal((B_GLOB, CIN, H, W), dtype=np.float32)
    W_ = (rng.standard_normal((M, D), dtype=np.float32) / np.sqrt(D)).astype(np.float32)
    gamma = np.ones((1,), np.float32)
    beta = np.zeros((COUT,), np.float32); beta[0] = 1.0
    out = kernel(x=x, W=W_, gamma=gamma, beta=beta)
    print("out", out.shape, out.dtype, np.abs(out).max())



# revision 2
# speedup vs baseline: 1.6186x; 1.6186x over previous
"""Trainium2 Bass kernel for nn_EuclideanToLorentzConv (8-core data-parallel).

v2 — bf16 matmul path + big-DMA I/O:
  * Conv as 9 window-matmuls in bf16 (fp32 matmul = 2 PE passes; bf16 is ~2.4x
    the streaming rate), reading a padded SBUF-resident [128,114,114] image
    XP = [s | s^2] built once per image via DVE/ACT casts (no per-band DMA).
  * Per-pixel stats (w0dot, T^2-1, sum y^2, mudot, stsq, rsq) via tiny PE
    matmuls; all transcendental math runs in fp32 on [56,448] pixel-major
    tiles as before.
  * y' lives in SBUF as bf16 [128, 25088]; output written per 2-band chunk
    as [127, 3584] fp32 with 50KB-contiguous-per-partition DMAs (127
    descriptors instead of ~500 per 4-row group).
  * Lorentz batchnorm statistics via two tiny AllReduces (unchanged).
"""

import sys
import numpy as np
from contextlib import ExitStack

sys.path.insert(0, "/opt/trn_rl_repo")

import concourse.bass as bass  # noqa: E402
import concourse.tile as tile  # noqa: E402
from concourse import mybir, bacc  # noqa: E402
from concourse.bass_utils import run_bass_kernel_spmd  # noqa: E402

F32 = mybir.dt.float32
BF16 = mybir.dt.bfloat16
AX = mybir.AxisListType
OP = mybir.AluOpType
AF = mybir.ActivationFunctionType

# ---- problem constants (hardcoded; kernel.py must be self-contained) ----
NCORES = 8
B_GLOB, CIN, H, W = 16, 64, 112, 112
B_LOC = B_GLOB // NCORES            # 2 images per core
S = CIN - 1                         # 63 space channels in
M = 127                             # space channels out
COUT = M + 1
D = 9 * S + 1                       # 568
EPS = 1e-6

HP, WP = H + 2, W + 2               # padded 114x114
ROWS_PER_GROUP = 4
GROUP_PX = ROWS_PER_GROUP * W       # 448
BAND_ROWS = 16                      # output rows per band
GROUPS_PER_BAND = BAND_ROWS // ROWS_PER_GROUP   # 4
BANDS_PER_IMG = H // BAND_ROWS      # 7
NBANDS = B_LOC * BANDS_PER_IMG      # 14
NGROUPS = NBANDS * GROUPS_PER_BAND  # 56
NPX = NGROUPS * GROUP_PX            # 25088 pixels per core
NPX_GLOB = B_GLOB * H * W           # 200704
IMG_PX = H * W                      # 12544

CHUNK_ROWS = 14                     # x staging chunk (rows per chunk)
CHUNKS_PER_IMG = H // CHUNK_ROWS    # 8
OUT_BANDS = 2                       # output chunk = 2 bands (32 rows)
OUT_COLS = OUT_BANDS * BAND_ROWS * W  # 3584

_CACHE = {}


def _build_nc():
    nc = bacc.Bacc("TRN2", target_bir_lowering=False, debug=False,
                   num_devices=NCORES)

    x_in = nc.dram_tensor("x", [B_LOC, CIN, H, W], F32, kind="ExternalInput")
    w9_in = nc.dram_tensor("w9", [128, 9 * 128], BF16, kind="ExternalInput")
    sw2_in = nc.dram_tensor("sw2", [128, 2], BF16, kind="ExternalInput")
    redw_in = nc.dram_tensor("redw", [M, 3], F32, kind="ExternalInput")
    lr1_in = nc.dram_tensor("lr1i", [2, 128], BF16, kind="ExternalInput")
    gamma_in = nc.dram_tensor("gamma", [1], F32, kind="ExternalInput")
    out_d = nc.dram_tensor("out", [B_LOC, COUT, H, W], F32,
                           kind="ExternalOutput")

    if _CACHE.get("debug"):
        dbg_ycm = nc.dram_tensor("dbg_ycm", [128, NPX], BF16, kind="ExternalOutput")
        dbg_ps1 = nc.dram_tensor("dbg_ps1", [5, NGROUPS, GROUP_PX], F32, kind="ExternalOutput")
        dbg_ps2 = nc.dram_tensor("dbg_ps2", [6, NGROUPS, GROUP_PX], F32, kind="ExternalOutput")
        dbg_mu = nc.dram_tensor("dbg_mu", [130], F32, kind="ExternalOutput")
        dbg_tmp = nc.dram_tensor("dbg_tmp", [128, NPX], BF16, kind="ExternalOutput")
    cc1_in = nc.dram_tensor("cc1_in", [130], F32)
    cc1_out = nc.dram_tensor("cc1_out", [130], F32, addr_space="Shared")
    cc2_in = nc.dram_tensor("cc2_in", [2], F32)
    cc2_out = nc.dram_tensor("cc2_out", [2], F32, addr_space="Shared")
    groups_all = [list(range(NCORES))]

    with tile.TileContext(nc) as tc, ExitStack() as ctx:
        sing = ctx.enter_context(tc.tile_pool(name="sing", bufs=1))
        stagep = ctx.enter_context(tc.tile_pool(name="stage", bufs=2))
        ysqp = ctx.enter_context(tc.tile_pool(name="ysq", bufs=2))
        outp = ctx.enter_context(tc.tile_pool(name="outp", bufs=2))
        stgp = ctx.enter_context(tc.tile_pool(name="stg", bufs=2))
        psy = ctx.enter_context(tc.tile_pool(name="psy", bufs=3, space="PSUM"))
        pss = ctx.enter_context(tc.tile_pool(name="pss", bufs=2, space="PSUM"))
        psb = ctx.enter_context(tc.tile_pool(name="psb", bufs=2, space="PSUM"))
        pst = ctx.enter_context(tc.tile_pool(name="pst", bufs=1, space="PSUM"))

        # ---- static SBUF ----
        W9B = sing.tile([128, 9, 128], BF16)
        nc.sync.dma_start(out=W9B, in_=w9_in[:].rearrange("p (w m) -> p w m", w=9))
        SW2 = sing.tile([128, 2], BF16)
        nc.sync.dma_start(out=SW2, in_=sw2_in[:])
        REDW = sing.tile([M, 3], F32)
        nc.sync.dma_start(out=REDW, in_=redw_in[:])
        LR1B = sing.tile([2, 128], BF16)
        nc.sync.dma_start(out=LR1B, in_=lr1_in[:])
        GAM = sing.tile([1, 1], F32)
        nc.sync.dma_start(out=GAM, in_=gamma_in[:].rearrange("(o c) -> o c", o=1))
        ONE127 = sing.tile([M, 1], BF16)
        nc.vector.memset(ONE127, 1.0)
        ONEROW = sing.tile([1, M], BF16)
        nc.vector.memset(ONEROW, 1.0)
        ONES56 = sing.tile([56, 1], F32)
        nc.vector.memset(ONES56, 1.0)
        BYT = sing.tile([56, 1], F32)
        nc.vector.memset(BYT, float(1.0 + _CACHE["c_w0sq"]))
        BM1 = sing.tile([56, 1], F32)
        nc.vector.memset(BM1, -1.0)
        BEPSV = sing.tile([1, 1], F32)
        nc.vector.memset(BEPSV, 1e-5)

        YCMB = sing.tile([128, NPX], BF16)        # rows 0..126 y', row 127 T^2-1
        MUP = sing.tile([128, NGROUPS], F32)      # per-group per-channel sums
        XP = sing.tile([128, HP, WP], BF16)       # [s | s^2] padded image
        nc.vector.memset(XP, 0.0)

        # pixel-scalar fields, [56, 448] (partition = group)
        def ps(name, dt=F32):
            t = sing.tile([NGROUPS, GROUP_PX], dt, tag=name, name=name)
            return t
        T2M1, TPS, W0DOT, YSQ1, YT = ps("t2m1"), ps("tps"), ps("w0dot"), ps("ysq1"), ps("yt")
        MUDOT, ALPHA, FPS, HPS = ps("mudot"), ps("alpha"), ps("fps"), ps("hps")
        STSQ, RSQ2, PSA, PSB, PSC = ps("stsq"), ps("rsq2"), ps("psa"), ps("psb"), ps("psc")
        TPSB, HPSB, W2B = ps("tpsb", BF16), ps("hpsb", BF16), ps("w2b", BF16)

        # ================= PHASE 1: conv =================
        with nc.allow_low_precision("bf16 conv by design"):
            for b in range(B_LOC):
                # build XP = [s | s^2] bf16 with padding
                for q in range(CHUNKS_PER_IMG):
                    r0 = q * CHUNK_ROWS
                    stg = stagep.tile([128, CHUNK_ROWS, W], F32, tag="stg")
                    src = x_in[b, 1:CIN, r0:r0 + CHUNK_ROWS, :].rearrange(
                        "c h w -> c (h w)")
                    nc.sync.dma_start(
                        out=stg[0:S].rearrange("c h w -> c (h w)"), in_=src)
                    nc.gpsimd.dma_start(
                        out=stg[64:64 + S].rearrange("c h w -> c (h w)"), in_=src)
                    nc.vector.tensor_scalar_add(
                        XP[0:S, 1 + r0:1 + r0 + CHUNK_ROWS, 1:1 + W],
                        stg[0:S], 0.0)
                    nc.scalar.activation(
                        out=XP[64:64 + S, 1 + r0:1 + r0 + CHUNK_ROWS, 1:1 + W],
                        in_=stg[64:64 + S], func=AF.Square)

                for rb in range(BANDS_PER_IMG):
                    band = b * BANDS_PER_IMG + rb
                    STGA = stgp.tile([2, GROUPS_PER_BAND, GROUP_PX], F32, tag="stga")
                    STGB = stgp.tile([1, GROUPS_PER_BAND, GROUP_PX], F32, tag="stg1")
                    for k in range(GROUPS_PER_BAND):
                        g = band * GROUPS_PER_BAND + k
                        cols = bass.ts(g, GROUP_PX)
                        h0 = rb * BAND_ROWS + k * ROWS_PER_GROUP
                        psum = psy.tile([128, GROUP_PX], F32, tag="psy")
                        for wi in range(9):
                            i, j = divmod(wi, 3)
                            rhs = XP[:, h0 + i:h0 + i + ROWS_PER_GROUP, j:j + W]
                            nc.tensor.matmul(psum[:], lhsT=W9B[:, wi, :], rhs=rhs,
                                             start=(wi == 0), stop=(wi == 8))
                        # evacuate to bf16 + per-channel partial sums (for mu)
                        nc.vector.tensor_scalar(out=YCMB[:, cols], in0=psum[:],
                                                scalar1=0.0, scalar2=None, op0=OP.add,
                                                op1=OP.add, accum_out=MUP[:, g:g + 1])
                        ysq = ysqp.tile([M, GROUP_PX], BF16, tag="ysq")
                        nc.scalar.activation(out=ysq, in_=psum[0:M, :], func=AF.Square)
                        psA = pss.tile([2, GROUP_PX], F32, tag="pss")
                        nc.tensor.matmul(psA[:], lhsT=SW2, rhs=YCMB[:, cols],
                                         start=True, stop=True)
                        psB = psb.tile([1, GROUP_PX], F32, tag="psb")
                        nc.tensor.matmul(psB[:], lhsT=ONE127, rhs=ysq[:],
                                         start=True, stop=True)
                        nc.vector.tensor_copy(out=STGA[:, k, :], in_=psA[:])
                        nc.vector.tensor_copy(out=STGB[:, k, :], in_=psB[:])
                    gsl = bass.ts(band, GROUPS_PER_BAND)
                    nc.sync.dma_start(out=W0DOT[gsl, :], in_=STGA[0:1, :, :])
                    nc.sync.dma_start(out=T2M1[gsl, :], in_=STGA[1:2, :, :])
                    nc.scalar.dma_start(out=YSQ1[gsl, :], in_=STGB[0:1, :, :])

        # ---- pixel-scalar chain, phase 1 ----
        # T = sqrt(1 + T2m1)
        nc.scalar.activation(out=TPS, in_=T2M1, func=AF.Sqrt, bias=1.0)
        # ysqf = ysq1 + 2*T*w0dot + T2m1*c_w0sq ; y_t = sqrt(1 + c_w0sq + ysqf')
        nc.vector.tensor_mul(PSA, TPS, W0DOT)
        nc.vector.scalar_tensor_tensor(out=PSB, in0=PSA, scalar=2.0, in1=YSQ1,
                                       op0=OP.mult, op1=OP.add)
        nc.vector.scalar_tensor_tensor(out=PSC, in0=T2M1, scalar=_CACHE["c_w0sq"],
                                       in1=PSB, op0=OP.mult, op1=OP.add)
        nc.scalar.activation(out=YT, in_=PSC, func=AF.Sqrt, bias=BYT[:])
        # reduced sums for the collective
        MUS = sing.tile([128, 1], F32)
        nc.vector.tensor_reduce(MUS, MUP, axis=AX.X, op=OP.add)
        SR = sing.tile([56, 2], F32)
        nc.vector.tensor_reduce(SR[:, 0:1], TPS, axis=AX.X, op=OP.add)
        nc.vector.tensor_reduce(SR[:, 1:2], YT, axis=AX.X, op=OP.add)
        pt = pst.tile([1, 8], F32, tag="pst")
        nc.tensor.matmul(pt[0:1, 0:2], lhsT=ONES56, rhs=SR[:], start=True, stop=True)
        SC0 = sing.tile([1, 2], F32)
        nc.vector.tensor_copy(out=SC0, in_=pt[0:1, 0:2])
        nc.sync.dma_start(out=cc1_in[0:128], in_=MUS)
        nc.sync.dma_start(out=cc1_in[128:130], in_=SC0)
        nc.gpsimd.collective_compute("AllReduce", OP.add, replica_groups=groups_all,
                                     ins=[cc1_in[:]], outs=[cc1_out[:]])
        MUSG = sing.tile([128, 1], F32)
        nc.sync.dma_start(out=MUSG, in_=cc1_out[0:128].rearrange("(p o) -> p o", o=1))
        SC0G = sing.tile([1, 2], F32)
        nc.sync.dma_start(out=SC0G, in_=cc1_out[128:130].rearrange("(o c) -> o c", o=1))

        # ---- mu normalization (tiny ops) ----
        invN = 1.0 / float(NPX_GLOB)
        SC127 = sing.tile([M, 2], F32)
        nc.gpsimd.partition_broadcast(SC127, SC0G)
        MUUS = sing.tile([M, 1], F32)      # unnormalized mean of y_s
        nc.vector.scalar_tensor_tensor(out=MUUS, in0=REDW[:, 0:1],
                                       scalar=SC127[:, 0:1], in1=MUSG[0:M, :],
                                       op0=OP.mult, op1=OP.add)
        nc.vector.tensor_scalar_mul(MUUS, MUUS, invN)
        MU0U = sing.tile([1, 1], F32)
        nc.vector.tensor_scalar_mul(MU0U, SC0G[0:1, 1:2], invN)
        MSQ = sing.tile([M, 1], F32)
        nc.vector.tensor_mul(MSQ, MUUS, MUUS)
        pt2 = pst.tile([1, 8], F32, tag="pst")
        nc.tensor.matmul(pt2[0:1, 0:1], lhsT=REDW[:, 2:3], rhs=MSQ[:],
                         start=True, stop=True)
        SMSQ = sing.tile([1, 1], F32)
        nc.vector.tensor_copy(out=SMSQ, in_=pt2[0:1, 0:1])
        T1 = sing.tile([1, 1], F32)
        nc.vector.tensor_mul(T1, MU0U, MU0U)
        nc.vector.tensor_sub(T1, T1, SMSQ)
        nc.scalar.activation(out=T1, in_=T1, func=AF.Sqrt)     # nrm
        RNRM = sing.tile([1, 1], F32)
        nc.vector.reciprocal(RNRM, T1)
        RN127 = sing.tile([M, 1], F32)
        nc.gpsimd.partition_broadcast(RN127, RNRM)
        MUHS = sing.tile([M, 1], F32)
        nc.vector.tensor_scalar_mul(MUHS, MUUS, RN127[:, 0:1])
        MU0H = sing.tile([1, 1], F32)
        nc.vector.tensor_mul(MU0H, MU0U, RNRM)
        # c_muW0 = sum(mu_s * W0)
        PRD = sing.tile([M, 1], F32)
        nc.vector.tensor_mul(PRD, MUHS, REDW[:, 0:1])
        pt3 = pst.tile([1, 8], F32, tag="pst")
        nc.tensor.matmul(pt3[0:1, 0:1], lhsT=REDW[:, 2:3], rhs=PRD[:],
                         start=True, stop=True)
        # inv1p = 1/(1+mu0)
        INV1P = sing.tile([1, 1], F32)
        nc.vector.tensor_scalar_add(INV1P, MU0H, 1.0)
        nc.vector.reciprocal(INV1P, INV1P)
        # scalar bundle -> 56 partitions: {mu0, inv1p, c_muW0}
        SCROW = sing.tile([1, 4], F32)
        nc.vector.tensor_copy(out=SCROW[:, 0:1], in_=MU0H)
        nc.vector.tensor_copy(out=SCROW[:, 1:2], in_=INV1P)
        nc.vector.tensor_copy(out=SCROW[:, 2:3], in_=pt3[0:1, 0:1])
        SC56 = sing.tile([56, 4], F32)
        nc.gpsimd.partition_broadcast(SC56, SCROW)
        # bf16 casts of mu for phase-2 matmuls
        with nc.allow_low_precision("bf16 mu by design"):
            MUHSB = sing.tile([M, 1], BF16)
            nc.vector.tensor_scalar_mul(MUHSB, MUHS, 1.0)
            NMUB = sing.tile([M, 1], BF16)
            nc.vector.tensor_scalar_mul(NMUB, MUHS, -1.0)
        # LR1B row0 = -mu_s  (tiny transposing DMA [127,1] -> [1,127])
        nc.sync.dma_start(out=LR1B[0:1, 0:M], in_=NMUB[:])

        if _CACHE.get("debug"):
            nc.sync.dma_start(out=dbg_ycm[:], in_=YCMB[:])
            for i_, t_ in enumerate([T2M1, TPS, W0DOT, YSQ1, YT]):
                nc.sync.dma_start(out=dbg_ps1[i_], in_=t_[:])
            nc.sync.dma_start(out=dbg_mu[0:128], in_=MUSG[:])
            nc.sync.dma_start(out=dbg_mu[128:130], in_=SC0G[:])

        # ================= PHASE 2 =================
        with nc.allow_low_precision("bf16 phase2 by design"):
            for band in range(NBANDS):
                STGC = stgp.tile([1, GROUPS_PER_BAND, GROUP_PX], F32, tag="stg1")
                for k in range(GROUPS_PER_BAND):
                    g = band * GROUPS_PER_BAND + k
                    cols = bass.ts(g, GROUP_PX)
                    psm = psb.tile([1, GROUP_PX], F32, tag="psb")
                    nc.tensor.matmul(psm[:], lhsT=MUHSB, rhs=YCMB[0:M, cols],
                                     start=True, stop=True)
                    nc.vector.tensor_copy(out=STGC[:, k, :], in_=psm[:])
                nc.sync.dma_start(out=MUDOT[bass.ts(band, GROUPS_PER_BAND), :],
                                  in_=STGC[0:1, :, :])

            # alpha = clip(mu0*yt - (mudot + T*c_muW0), 1+eps)
            nc.vector.scalar_tensor_tensor(out=PSA, in0=TPS, scalar=SC56[:, 2:3],
                                           in1=MUDOT, op0=OP.mult, op1=OP.add)
            nc.vector.tensor_scalar(out=PSB, in0=YT, scalar1=SC56[:, 0:1],
                                    scalar2=None, op0=OP.mult)
            nc.vector.tensor_sub(ALPHA, PSB, PSA)
            nc.vector.tensor_scalar_max(ALPHA, ALPHA, 1.0 + EPS)
            # f = ln(alpha + sqrt(alpha^2-1)) / sqrt(alpha^2-1)
            nc.vector.tensor_mul(PSA, ALPHA, ALPHA)
            nc.scalar.activation(out=PSB, in_=PSA, func=AF.Sqrt, bias=BM1[:])
            nc.vector.tensor_add(PSC, ALPHA, PSB)
            nc.scalar.activation(out=PSC, in_=PSC, func=AF.Ln)
            nc.vector.reciprocal(PSB, PSB)
            nc.vector.tensor_mul(FPS, PSC, PSB)
            # H = alpha + (yt - alpha*mu0) * inv1p
            nc.vector.tensor_scalar(out=PSA, in0=ALPHA, scalar1=SC56[:, 0:1],
                                    scalar2=None, op0=OP.mult)
            nc.vector.tensor_sub(PSA, YT, PSA)
            nc.vector.scalar_tensor_tensor(out=HPS, in0=PSA, scalar=SC56[:, 1:2],
                                           in1=ALPHA, op0=OP.mult, op1=OP.add)
            nc.scalar.activation(out=TPSB, in_=TPS, func=AF.Copy)
            nc.scalar.activation(out=HPSB, in_=HPS, func=AF.Copy)

            # tmp = y' + W0*T - mu_s*H ; stsq = sum_c tmp^2
            for band in range(NBANDS):
                gsl = bass.ts(band, GROUPS_PER_BAND)
                HT = stgp.tile([2, GROUPS_PER_BAND, GROUP_PX], BF16, tag="htb")
                nc.sync.dma_start(out=HT[0:1, :, :], in_=HPSB[gsl, :])
                nc.scalar.dma_start(out=HT[1:2, :, :], in_=TPSB[gsl, :])
                STGD = stgp.tile([1, GROUPS_PER_BAND, GROUP_PX], F32, tag="stg1")
                for k in range(GROUPS_PER_BAND):
                    g = band * GROUPS_PER_BAND + k
                    cols = bass.ts(g, GROUP_PX)
                    pr1 = psy.tile([128, GROUP_PX], F32, tag="psy")
                    nc.tensor.matmul(pr1[:], lhsT=LR1B, rhs=HT[:, k, :],
                                     start=True, stop=True)
                    nc.vector.tensor_add(YCMB[:, cols], YCMB[:, cols], pr1[:])
                    sq2 = ysqp.tile([M, GROUP_PX], BF16, tag="ysq")
                    nc.scalar.activation(out=sq2, in_=YCMB[0:M, cols], func=AF.Square)
                    psq = psb.tile([1, GROUP_PX], F32, tag="psb")
                    nc.tensor.matmul(psq[:], lhsT=ONE127, rhs=sq2[:],
                                     start=True, stop=True)
                    nc.vector.tensor_copy(out=STGD[:, k, :], in_=psq[:])
                nc.sync.dma_start(out=STSQ[gsl, :], in_=STGD[0:1, :, :])

        if _CACHE.get("debug"):
            nc.sync.dma_start(out=dbg_tmp[:], in_=YCMB[:])
            for i_, t_ in enumerate([MUDOT, ALPHA, FPS, HPS, STSQ]):
                nc.sync.dma_start(out=dbg_ps2[i_], in_=t_[:])

        # var = mean(f^2 * stsq)  -> allreduce
        nc.vector.tensor_mul(PSA, FPS, FPS)
        nc.vector.tensor_mul(PSB, PSA, STSQ)
        VR = sing.tile([56, 1], F32)
        nc.vector.tensor_reduce(VR, PSB, axis=AX.X, op=OP.add)
        pt4 = pst.tile([1, 8], F32, tag="pst")
        nc.tensor.matmul(pt4[0:1, 0:1], lhsT=ONES56, rhs=VR[:], start=True, stop=True)
        VSC = sing.tile([1, 2], F32)
        nc.vector.tensor_copy(out=VSC[:, 0:1], in_=pt4[0:1, 0:1])
        nc.vector.tensor_copy(out=VSC[:, 1:2], in_=pt4[0:1, 0:1])
        nc.sync.dma_start(out=cc2_in[:], in_=VSC)
        nc.gpsimd.collective_compute("AllReduce", OP.add, replica_groups=groups_all,
                                     ins=[cc2_in[:]], outs=[cc2_out[:]])
        VG = sing.tile([1, 2], F32)
        nc.sync.dma_start(out=VG, in_=cc2_out[:].rearrange("(o c) -> o c", o=1))
        GSC = sing.tile([1, 1], F32)
        nc.vector.tensor_scalar_mul(GSC, VG[0:1, 0:1], invN)
        nc.scalar.activation(out=GSC, in_=GSC, func=AF.Sqrt, bias=BEPSV[:])
        nc.vector.reciprocal(GSC, GSC)
        nc.vector.tensor_mul(GSC, GSC, GAM)
        G56 = sing.tile([56, 1], F32)
        nc.gpsimd.partition_broadcast(G56, GSC)

        # ================= PHASE 3 =================
        # gf = g*f ; vn = sqrt(max(gf^2*stsq, eps)); w2 = gf*sinh(vn)/vn
        nc.vector.tensor_scalar(out=PSA, in0=FPS, scalar1=G56[:, 0:1],
                                scalar2=None, op0=OP.mult)          # gf
        nc.vector.tensor_mul(PSB, PSA, PSA)
        nc.vector.tensor_mul(PSB, PSB, STSQ)
        nc.vector.tensor_scalar_max(PSB, PSB, EPS)
        VN = TPS
        nc.scalar.activation(out=VN, in_=PSB, func=AF.Sqrt)
        EX = W0DOT
        nc.scalar.activation(out=EX, in_=VN, func=AF.Exp)
        EIX = YSQ1
        nc.vector.reciprocal(EIX, EX)
        nc.vector.tensor_sub(PSB, EX, EIX)                          # 2*sinh
        nc.vector.reciprocal(PSC, VN)
        nc.vector.tensor_mul(PSB, PSB, PSC)
        nc.vector.tensor_scalar_mul(PSB, PSB, 0.5)                  # sinh/vn
        W2 = MUDOT
        nc.vector.tensor_mul(W2, PSA, PSB)
        with nc.allow_low_precision("bf16 w2 by design"):
            nc.scalar.activation(out=W2B, in_=W2, func=AF.Copy)
        if _CACHE.get("debug"):
            nc.sync.dma_start(out=dbg_ps2[5], in_=W2[:])

        out_flat = [out_d[b_, 1:COUT].rearrange("c h w -> c (h w)")
                    for b_ in range(B_LOC)]
        with nc.allow_low_precision("bf16 phase3 by design"):
            for b in range(B_LOC):
                for oc in range(BANDS_PER_IMG // OUT_BANDS + 1):   # 4 chunks: 2,2,2,1 bands
                    rb_lo = oc * OUT_BANDS
                    rb_hi = min(rb_lo + OUT_BANDS, BANDS_PER_IMG)
                    if rb_lo >= rb_hi:
                        continue
                    ncols = (rb_hi - rb_lo) * BAND_ROWS * W
                    OUTCH = outp.tile([M, OUT_COLS], F32, tag="outch")
                    for rb in range(rb_lo, rb_hi):
                        band = b * BANDS_PER_IMG + rb
                        gsl = bass.ts(band, GROUPS_PER_BAND)
                        W2S = stgp.tile([1, GROUPS_PER_BAND, GROUP_PX], BF16, tag="w2s")
                        nc.sync.dma_start(out=W2S[0:1, :, :], in_=W2B[gsl, :])
                        STGE = stgp.tile([1, GROUPS_PER_BAND, GROUP_PX], F32, tag="stg1")
                        for k in range(GROUPS_PER_BAND):
                            g = band * GROUPS_PER_BAND + k
                            cols = bass.ts(g, GROUP_PX)
                            off = (rb - rb_lo) * BAND_ROWS * W + k * GROUP_PX
                            psw2 = psy.tile([128, GROUP_PX], F32, tag="psy")
                            nc.tensor.matmul(psw2[0:M, :], lhsT=ONEROW,
                                             rhs=W2S[0:1, k, :], start=True, stop=True)
                            nc.vector.scalar_tensor_tensor(
                                out=OUTCH[:, off:off + GROUP_PX],
                                in0=YCMB[0:M, cols], scalar=0.0, in1=psw2[0:M, :],
                                op0=OP.max, op1=OP.mult)
                            sqo = ysqp.tile([M, GROUP_PX], BF16, tag="ysq")
                            nc.scalar.activation(out=sqo, in_=OUTCH[:, off:off + GROUP_PX],
                                                 func=AF.Square)
                            psr = psb.tile([1, GROUP_PX], F32, tag="psb")
                            nc.tensor.matmul(psr[:], lhsT=ONE127, rhs=sqo[:],
                                             start=True, stop=True)
                            nc.vector.tensor_copy(out=STGE[:, k, :], in_=psr[:])
                        nc.scalar.dma_start(out=RSQ2[gsl, :], in_=STGE[0:1, :, :])
                    col0 = rb_lo * BAND_ROWS * W
                    eng = nc.sync if (oc % 2 == 0) else nc.scalar
                    eng.dma_start(out=out_flat[b][:, col0:col0 + ncols],
                                  in_=OUTCH[:, 0:ncols])

        # rt = sqrt(1 + sum rs^2) -> channel 0 plane
        RT = T2M1
        nc.scalar.activation(out=RT, in_=RSQ2, func=AF.Sqrt, bias=1.0)
        nc.sync.dma_start(out=out_d[:, 0, :, :], in_=RT)

    nc.compile()
    return nc


def _prep_consts(W):
    W = np.asarray(W, np.float32)
    bf = mybir.dt.np(BF16)
    w9 = np.zeros((128, 9, 128), np.float32)
    for wi in range(9):
        w9[0:S, wi, 0:M] = W[:, 1 + wi * S:1 + (wi + 1) * S].T
        w9[64:64 + S, wi, 127] = 1.0
    sw2 = np.zeros((128, 2), np.float32)
    sw2[0:M, 0] = W[:, 0]
    sw2[127, 1] = 1.0
    redw = np.zeros((M, 3), np.float32)
    redw[:, 0] = W[:, 0]
    redw[:, 2] = 1.0
    lr1 = np.zeros((2, 128), np.float32)
    lr1[1, 0:M] = W[:, 0]
    c_w0sq = float(np.float32((W[:, 0].astype(np.float64) ** 2).sum()))
    return (w9.reshape(128, 9 * 128).astype(bf), sw2.astype(bf), redw,
            lr1.astype(bf), c_w0sq)


def _in_maps(x, W, gamma):
    x = np.ascontiguousarray(np.asarray(x, np.float32))
    gamma = np.asarray(gamma, np.float32)
    w9, sw2, redw, lr1, c_w0sq = _prep_consts(W)
    if "nc" not in _CACHE:
        _CACHE["c_w0sq"] = c_w0sq
        _CACHE["nc"] = _build_nc()
    maps = []
    for c in range(NCORES):
        maps.append({
            "x": x[c * B_LOC:(c + 1) * B_LOC],
            "w9": w9, "sw2": sw2, "redw": redw, "lr1i": lr1,
            "gamma": gamma,
        })
    return _CACHE["nc"], maps


def kernel(x, W, gamma, beta):
    beta = np.asarray(beta, np.float32)
    gamma = np.asarray(gamma, np.float32)
    assert abs(float(beta[0]) - 1.0) < 1e-6 and np.all(np.abs(beta[1:]) < 1e-6), \
        "kernel specialized for beta == Lorentz origin"
    assert float(gamma[0]) > 0.0
    nc, in_maps = _in_maps(x, W, gamma)
    res = run_bass_kernel_spmd(nc, in_maps, list(range(NCORES)))
    out = np.concatenate([res.results[c]["out"] for c in range(NCORES)], axis=0)
    return out


def run_traced(inputs, tmpdir=None):
    """Run with NTFF tracing; returns (exec_time_ns, BassKernelResults)."""
    nc, in_maps = _in_maps(inputs["x"], inputs["W"], inputs["gamma"])
    res = run_bass_kernel_spmd(nc, in_maps, list(range(NCORES)),
                               trace=True, tmpdir=tmpdir)
    return res.exec_time_ns, res


def simulate(inputs, debug=True):
    """Run the kernel through MultiCoreSim; returns list of per-core output dicts."""
    from concourse.bass_interp import MultiCoreSim
    _CACHE.clear()
    _CACHE["debug"] = debug
    x = np.asarray(inputs["x"], np.float32)
    w9, sw2, redw, lr1, c_w0sq = _prep_consts(inputs["W"])
    _CACHE["c_w0sq"] = c_w0sq
    nc = _build_nc()
    sim = MultiCoreSim(nc, num_cores=NCORES)
    for c in range(NCORES):
        cs = sim.cores[c]
        cs.tensor("x")[:] = x[c * B_LOC:(c + 1) * B_LOC]
        cs.tensor("w9")[:] = w9
        cs.tensor("sw2")[:] = sw2
        cs.tensor("redw")[:] = redw
        cs.tensor("lr1i")[:] = lr1
        cs.tensor("gamma")[:] = np.asarray(inputs["gamma"], np.float32)
    sim.simulate(check_with_hw=False)
    names = ["out"]
    if debug:
        names += ["dbg_ycm", "dbg_ps1", "dbg_ps2", "dbg_mu", "dbg_tmp"]
    return [{n: np.array(sim.cores[c].tensor(n)) for n in names}
            for c in range(NCORES)]


if __name__ == "__main__":
    rng = np.random.default_rng(0)
    x = rng.standard_normal((B_GLOB, CIN, H, W), dtype=np.float32)
    W_ = (rng.standard_normal((M, D), dtype=np.float32) / np.sqrt(D)).astype(np.float32)
    gamma = np.ones((1,), np.float32)
    beta = np.zeros((COUT,), np.float32); beta[0] = 1.0
    out = kernel(x=x, W=W_, gamma=gamma, beta=beta)
    print("out", out.shape, out.dtype, np.abs(out).max())
